# revision 15
# baseline (speedup 1.0000x reference)
"""MinGRU block kernel for Trainium2 (8 NeuronCores, batch-parallel).

Reference computation (per batch sample, sharded one sample per core):
  xn = LayerNorm(x) * gamma + beta
  gh = xn @ W_gh ;  k = gh[:, :H], h_pre = gh[:, H:]
  log-space Heinsen scan == direct linear recurrence (exact algebraic identity):
      z = sigmoid(k); g = where(h_pre>=0, h_pre+0.5, sigmoid(h_pre))
        = max(h_pre+0.5, sigmoid(h_pre))          # exact: curves cross only at 0
      h_t = (1-z_t) h_{t-1} + z_t g_t
  out = h + x ; hidden = h[:, -1]

Device mapping per core:
  - LayerNorm stats via bn_stats/bn_aggr (DVE), rsqrt via sqrt+reciprocal+Newton
  - xn cast to bf16 (GPSIMD), transposed 128x128 on PE into [d, t] layout
  - GEMM W'^T @ xn^T in bf16 (W' = gamma-folded W), K=4x128 PSUM accumulation
  - c = sigmoid(-k) (ACT from PSUM), g = max(psum+0.5, s) fused stt (DVE)
  - v' = (c-1)*g (DVE); hardware scan: state = (c * state) - v'  (DVE, f32 out)
  - back-transpose h on PE (f32), residual add from PSUM (DVE), DMA out
gamma/beta are folded host-side: W' = gamma[:,None]*W, s2 = beta @ W rides the
activation biases (zero for the reference inputs, enabling 2-bank fused ops).
Host pre-permutes x/out/W DRAM layouts so DMAs are contiguous per partition.
"""

import numpy as np
import ml_dtypes

import concourse.bass as bass
import concourse.bacc as bacc
import concourse.mybir as mybir
import concourse.tile as tile
from concourse.bass_utils import run_bass_kernel_spmd

B, T, D, H = 8, 4096, 512, 512
NCORES = 8
NCHUNK = 8          # t-chunks per core
TC = T // NCHUNK    # 512 timesteps per chunk
NSUB = TC // 128    # 4 sub-tiles of 128 rows
NDC = D // 128      # 4 contraction chunks
NHC = (2 * H) // 128  # 8 output-channel chunks
F32 = mybir.dt.float32
BF16 = mybir.dt.bfloat16
LN_EPS = 1e-10
AF = mybir.ActivationFunctionType
OP = mybir.AluOpType


def _build(fused: bool):
    nc = bacc.Bacc("TRN2", target_bir_lowering=False, debug=False, num_devices=NCORES)
    # host-permuted layouts: x/out [c, p, i, d] with row t = c*512 + i*128 + p
    x_d = nc.declare_dram_parameter("x", [NCHUNK, 128, NSUB, TC], F32, isOutput=False)
    w_d = nc.declare_dram_parameter("w", [128, NDC, 2 * H], BF16, isOutput=False)
    b_d = nc.declare_dram_parameter("bias", [128, 12], F32, isOutput=False)
    id_d = nc.declare_dram_parameter("ident", [128, 128], BF16, isOutput=False)
    idf_d = nc.declare_dram_parameter("identf", [128, 128], F32, isOutput=False)
    out_d = nc.declare_dram_parameter(
        "out", [NCHUNK, 128, NSUB, TC], F32, isOutput=True
    )
    hid_d = nc.declare_dram_parameter("hidden", [H], F32, isOutput=True)

    with tile.TileContext(nc) as tc:
        with (
            tc.tile_pool(name="const", bufs=1) as pc,
            tc.tile_pool(name="px", bufs=NCHUNK) as px,
            tc.tile_pool(name="pstat", bufs=NCHUNK) as pstat,
            tc.tile_pool(name="ptmp", bufs=3) as ptmp,
            tc.tile_pool(name="pxn", bufs=3) as pxn,
            tc.tile_pool(name="pxnT", bufs=3) as pxnT,
            tc.tile_pool(name="pcv", bufs=3) as pcv,
            tc.tile_pool(name="ph", bufs=2) as ph,
            tc.tile_pool(name="pout", bufs=2) as pout,
            tc.tile_pool(name="ptr", bufs=2, space="PSUM") as ptr,
            tc.tile_pool(name="pmm", bufs=2, space="PSUM") as pmm,
            tc.tile_pool(name="pbk", bufs=2, space="PSUM") as pbk,
        ):
            # ---- Phase A: load all x, LayerNorm stats ----
            # (x chunk 0 is issued first, split per subtile, so DVE work can
            # start ~6us earlier; the 1MB weight load rides behind it)
            xs, stats = [], []
            w_sb = None
            for c in range(NCHUNK):
                x_sb = px.tile([128, NSUB, TC], F32, tag="x")
                if c == 0:
                    for i in range(NSUB):
                        nc.sync.dma_start(x_sb[:, i, :], x_d[c][:, i, :])
                else:
                    nc.sync.dma_start(x_sb[:].opt(), x_d[c].opt())
                if c == 1:
                    # consts: needed from the transpose/matmul stage onwards
                    id_sb = pc.tile([128, 128], BF16)
                    nc.sync.dma_start(id_sb[:], id_d[:])
                    w_sb = pc.tile([128, NDC, 2 * H], BF16)
                    nc.sync.dma_start(w_sb[:].opt(), w_d[:].opt())
                    if not fused:
                        bias_sb = pc.tile([128, 12], F32)
                        nc.sync.dma_start(bias_sb[:], b_d[:])
                mv = pstat.tile([128, 2, NSUB], F32, tag="mv")
                for i in range(NSUB):
                    bn6 = ptmp.tile([128, 6], F32, tag="bn6")
                    nc.vector.bn_stats(bn6[:], x_sb[:, i, :])
                    nc.vector.bn_aggr(mv[:, :, i], bn6[:])
                # rstd = 1/sqrt(var+eps), one Newton step to wash out the
                # ACT sqrt table's loose tolerance
                ve = ptmp.tile([128, NSUB], F32, tag="ve")
                nc.vector.tensor_scalar(ve[:], mv[:, 1, :], LN_EPS, None, OP.add)
                sq = ptmp.tile([128, NSUB], F32, tag="sq")
                nc.scalar.activation(sq[:], ve[:], AF.Sqrt)
                r0 = ptmp.tile([128, NSUB], F32, tag="r0")
                nc.vector.reciprocal(r0[:], sq[:])
                rr = ptmp.tile([128, NSUB], F32, tag="rr")
                nc.vector.tensor_tensor(rr[:], r0[:], r0[:], OP.mult)
                vr = ptmp.tile([128, NSUB], F32, tag="vr")
                nc.vector.tensor_tensor(vr[:], rr[:], ve[:], OP.mult)
                t3 = ptmp.tile([128, NSUB], F32, tag="t3")
                nc.vector.tensor_scalar(t3[:], vr[:], -0.5, 1.5, OP.mult, OP.add)
                r1 = pstat.tile([128, NSUB], F32, tag="r1")
                nc.vector.tensor_tensor(r1[:], t3[:], r0[:], OP.mult)
                xs.append(x_sb)
                stats.append((mv, r1))

            # ---- Phase B: per-chunk pipeline ----
            hTs = []
            for c in range(NCHUNK):
                x_sb = xs[c]
                mv, r1 = stats[c]

                xn = pxn.tile([128, NSUB, TC], BF16, tag="xn")
                for i in range(NSUB):
                    nc.gpsimd.tensor_scalar(
                        xn[:, i, :], x_sb[:, i, :],
                        mv[:, 0, i:i + 1], r1[:, i:i + 1],
                        OP.subtract, OP.mult,
                    )

                xnT = pxnT.tile([128, NDC, TC], BF16, tag="xnT")
                for dc in range(NDC):
                    pt = ptr.tile([128, TC], BF16, tag="ptr")
                    for i in range(NSUB):
                        nc.tensor.transpose(
                            pt[:, i * 128:(i + 1) * 128],
                            xn[:, i, dc * 128:(dc + 1) * 128],
                            id_sb[:],
                        )
                    nc.scalar.copy(xnT[:, dc, :], pt[:])

                cb = pcv.tile([128, 4, TC], BF16, tag="c")
                sb_ = pcv.tile([128, 4, TC], BF16, tag="s")
                gb = pcv.tile([128, 4, TC], BF16, tag="g")
                vp = pcv.tile([128, 4, TC], BF16, tag="vp")

                if fused:
                    # beta == 0: biases are constants, so post-matmul ops fuse
                    # across hc pairs ([128, 1024] spanning 2 PSUM banks)
                    for half in range(2):
                        km = pmm.tile([128, 2, TC], F32, tag="mm")
                        for u in range(2):
                            hc = 2 * half + u
                            for dc in range(NDC):
                                nc.tensor.matmul(
                                    km[:, u, :],
                                    w_sb[:, dc, hc * 128:(hc + 1) * 128],
                                    xnT[:, dc, :],
                                    start=(dc == 0), stop=(dc == NDC - 1),
                                )
                        sl = slice(2 * half, 2 * half + 2)
                        nc.scalar.activation(cb[:, sl, :], km[:], AF.Sigmoid,
                                             scale=-1.0)
                    for half in range(2):
                        hm = pmm.tile([128, 2, TC], F32, tag="mm")
                        for u in range(2):
                            hc = 4 + 2 * half + u
                            for dc in range(NDC):
                                nc.tensor.matmul(
                                    hm[:, u, :],
                                    w_sb[:, dc, hc * 128:(hc + 1) * 128],
                                    xnT[:, dc, :],
                                    start=(dc == 0), stop=(dc == NDC - 1),
                                )
                        sl = slice(2 * half, 2 * half + 2)
                        nc.scalar.activation(sb_[:, sl, :], hm[:], AF.Sigmoid)
                        nc.vector.scalar_tensor_tensor(
                            gb[:, sl, :], hm[:], 0.5, sb_[:, sl, :],
                            OP.add, OP.max,
                        )
                        nc.vector.scalar_tensor_tensor(
                            vp[:, sl, :], cb[:, sl, :], 1.0, gb[:, sl, :],
                            OP.subtract, OP.mult,
                        )
                else:
                    for hc in range(NHC):
                        ps = pmm.tile([128, 2, TC], F32, tag="mm")
                        for dc in range(NDC):
                            nc.tensor.matmul(
                                ps[:, 0, :],
                                w_sb[:, dc, hc * 128:(hc + 1) * 128],
                                xnT[:, dc, :],
                                start=(dc == 0), stop=(dc == NDC - 1),
                            )
                        j = hc % 4
                        if hc < 4:
                            nc.scalar.activation(
                                cb[:, j, :], ps[:, 0, :], AF.Sigmoid,
                                bias=bias_sb[:, j:j + 1], scale=-1.0,
                            )
                        else:
                            nc.scalar.activation(
                                sb_[:, j, :], ps[:, 0, :], AF.Sigmoid,
                                bias=bias_sb[:, 4 + j:5 + j], scale=1.0,
                            )
                            nc.vector.scalar_tensor_tensor(
                                gb[:, j, :], ps[:, 0, :], bias_sb[:, 8 + j:9 + j],
                                sb_[:, j, :], OP.add, OP.max,
                            )
                            nc.vector.scalar_tensor_tensor(
                                vp[:, j, :], cb[:, j, :], 1.0,
                                gb[:, j, :], OP.subtract, OP.mult,
                            )

                hT = ph.tile([128, 4, TC], BF16, tag="h")
                for j in range(4):
                    init = 0.0 if c == 0 else hTs[c - 1][:, j, TC - 1:TC]
                    nc.vector.tensor_tensor_scan(
                        hT[:, j, :], cb[:, j, :], vp[:, j, :], init,
                        op0=OP.mult, op1=OP.subtract,
                    )
                hTs.append(hT)

                outt = pout.tile([128, NSUB, TC], F32, tag="out")
                for grp in range(2):
                    pb = pbk.tile([128, 2, D], BF16, tag="bk")
                    for u in range(2):
                        i = 2 * grp + u
                        for j in range(4):
                            nc.tensor.transpose(
                                pb[:, u, j * 128:(j + 1) * 128],
                                hT[:, j, i * 128:(i + 1) * 128],
                                id_sb[:],
                            )
                    sl = slice(2 * grp, 2 * grp + 2)
                    nc.vector.tensor_tensor(outt[:, sl, :], pb[:], x_sb[:, sl, :],
                                            OP.add)
                    if c == NCHUNK - 1 and grp == 1:
                        # hidden = h[T-1] = partition 127 of the last
                        # back-transpose tile (before the residual add)
                        hid_sb = pc.tile([32, H], F32)
                        nc.scalar.copy(hid_sb[:], pb[96:128, 1, :])
                        nc.sync.dma_start(hid_d[:], hid_sb[31:32, :])
                    if c == NCHUNK - 1:
                        nc.sync.dma_start(out_d[c][:, sl, :].opt(),
                                          outt[:, sl, :].opt())
                if c < NCHUNK - 1:
                    nc.sync.dma_start(out_d[c].opt(), outt[:].opt())

    nc.compile()
    return nc


_PROGRAMS = {}


def _get_program(fused=True):
    if fused not in _PROGRAMS:
        _PROGRAMS[fused] = _build(fused)
    return _PROGRAMS[fused]


def _permute_x(xc):
    # [4096, 512] -> [NCHUNK, 128, NSUB, TC] with row t = c*512 + i*128 + p
    return np.ascontiguousarray(
        xc.reshape(NCHUNK, NSUB, 128, D).transpose(0, 2, 1, 3)
    )


def _unpermute_out(oc):
    # [NCHUNK, 128, NSUB, TC] -> [4096, 512]
    return oc.transpose(0, 2, 1, 3).reshape(T, D)


def _host_prep(x, ln_gamma, ln_beta, W_gh):
    x = np.ascontiguousarray(np.asarray(x, dtype=np.float32))
    g = np.asarray(ln_gamma, dtype=np.float32)
    be = np.asarray(ln_beta, dtype=np.float32)
    W = np.asarray(W_gh, dtype=np.float32)
    Wp = (g[:, None] * W).astype(ml_dtypes.bfloat16)
    Wp = np.ascontiguousarray(Wp.reshape(NDC, 128, 2 * H).transpose(1, 0, 2))
    s2 = (be @ W).astype(np.float32)  # [2H]
    bias = np.zeros((128, 12), dtype=np.float32)
    for j in range(4):
        bias[:, j] = -s2[j * 128:(j + 1) * 128]
        bias[:, 4 + j] = s2[H + j * 128: H + (j + 1) * 128]
        bias[:, 8 + j] = s2[H + j * 128: H + (j + 1) * 128] + 0.5
    ident = np.eye(128, dtype=ml_dtypes.bfloat16)
    identf = np.eye(128, dtype=np.float32)
    fused = bool(np.all(s2 == 0.0))
    return x, Wp, bias, ident, identf, fused


def kernel(x, ln_gamma, ln_beta, W_gh, _trace=False, _spmd_kwargs=None):
    x, Wp, bias, ident, identf, fused = _host_prep(x, ln_gamma, ln_beta, W_gh)
    nc = _get_program(fused)
    in_maps = [
        {"x": _permute_x(x[c]), "w": Wp, "bias": bias, "ident": ident,
         "identf": identf}
        for c in range(NCORES)
    ]
    kw = dict(_spmd_kwargs or {})
    if _trace:
        kw.setdefault("trace", True)
    res = run_bass_kernel_spmd(nc, in_maps, list(range(NCORES)), **kw)
    out = np.stack([_unpermute_out(res.results[c]["out"]) for c in range(NCORES)])
    hidden = np.stack([res.results[c]["hidden"] for c in range(NCORES)])
    if _trace:
        return (out, hidden), res
    return out, hidden


# revision 25
# speedup vs baseline: 1.0476x; 1.0476x over previous
"""MinGRU block kernel for Trainium2 (8 NeuronCores, batch-parallel).

Reference computation (per batch sample, sharded one sample per core):
  xn = LayerNorm(x) * gamma + beta
  gh = xn @ W_gh ;  k = gh[:, :H], h_pre = gh[:, H:]
  log-space Heinsen scan == direct linear recurrence (exact algebraic identity):
      z = sigmoid(k); g = where(h_pre>=0, h_pre+0.5, sigmoid(h_pre))
        = max(h_pre+0.5, sigmoid(h_pre))          # exact: curves cross only at 0
      h_t = (1-z_t) h_{t-1} + z_t g_t
  out = h + x ; hidden = h[:, -1]

Device mapping per core:
  - LayerNorm stats via bn_stats/bn_aggr (DVE), rsqrt via sqrt+reciprocal+Newton
  - xn cast to bf16 (GPSIMD), transposed 128x128 on PE into [d, t] layout
  - GEMM W'^T @ xn^T in bf16 (W' = gamma-folded W), K=4x128 PSUM accumulation
  - c = sigmoid(-k) (ACT from PSUM), g = max(psum+0.5, s) fused stt (DVE)
  - v' = (c-1)*g (DVE); hardware scan: state = (c * state) - v'  (DVE, f32 out)
  - back-transpose h on PE (f32), residual add from PSUM (DVE), DMA out
gamma/beta are folded host-side: W' = gamma[:,None]*W, s2 = beta @ W rides the
activation biases (zero for the reference inputs, enabling 2-bank fused ops).
Host pre-permutes x/out/W DRAM layouts so DMAs are contiguous per partition.
"""

import numpy as np
import ml_dtypes

import concourse.bass as bass
import concourse.bacc as bacc
import concourse.mybir as mybir
import concourse.tile as tile
from concourse.bass_utils import run_bass_kernel_spmd

B, T, D, H = 8, 4096, 512, 512
NCORES = 8
NCHUNK = 8          # t-chunks per core
TC = T // NCHUNK    # 512 timesteps per chunk
NSUB = TC // 128    # 4 sub-tiles of 128 rows
NDC = D // 128      # 4 contraction chunks
NHC = (2 * H) // 128  # 8 output-channel chunks
F32 = mybir.dt.float32
BF16 = mybir.dt.bfloat16
LN_EPS = 1e-10
AF = mybir.ActivationFunctionType
OP = mybir.AluOpType


def _build(fused: bool):
    nc = bacc.Bacc("TRN2", target_bir_lowering=False, debug=False, num_devices=NCORES)
    # host-permuted layouts: x/out [c, p, i, d] with row t = c*512 + i*128 + p
    x_d = nc.declare_dram_parameter("x", [NCHUNK, 128, NSUB, TC], F32, isOutput=False)
    w_d = nc.declare_dram_parameter("w", [128, NDC, 2 * H], BF16, isOutput=False)
    b_d = nc.declare_dram_parameter("bias", [128, 12], F32, isOutput=False)
    id_d = nc.declare_dram_parameter("ident", [128, 128], BF16, isOutput=False)
    idf_d = nc.declare_dram_parameter("identf", [128, 128], F32, isOutput=False)
    out_d = nc.declare_dram_parameter(
        "out", [NCHUNK, 128, NSUB, TC], F32, isOutput=True
    )
    hid_d = nc.declare_dram_parameter("hidden", [H], F32, isOutput=True)

    with tile.TileContext(nc) as tc:
        with (
            tc.tile_pool(name="const", bufs=1) as pc,
            tc.tile_pool(name="px", bufs=NCHUNK) as px,
            tc.tile_pool(name="pstat", bufs=NCHUNK) as pstat,
            tc.tile_pool(name="ptmp", bufs=3) as ptmp,
            tc.tile_pool(name="pxn", bufs=3) as pxn,
            tc.tile_pool(name="pxnT", bufs=3) as pxnT,
            tc.tile_pool(name="pcv", bufs=3) as pcv,
            tc.tile_pool(name="ph", bufs=3) as ph,
            tc.tile_pool(name="pout", bufs=2) as pout,
            tc.tile_pool(name="ptr", bufs=2, space="PSUM") as ptr,
            tc.tile_pool(name="pmm", bufs=2, space="PSUM") as pmm,
            tc.tile_pool(name="pbk", bufs=2, space="PSUM") as pbk,
        ):
            # ---- Phase A: load all x, LayerNorm stats ----
            # (x chunk 0 is issued first, split per subtile, so DVE work can
            # start ~6us earlier; the 1MB weight load rides behind it)
            xs, stats = [], []
            w_sb = None
            for c in range(NCHUNK):
                x_sb = px.tile([128, NSUB, TC], F32, tag="x")
                if c == 0:
                    for i in range(NSUB):
                        nc.sync.dma_start(x_sb[:, i, :], x_d[c][:, i, :])
                else:
                    nc.sync.dma_start(x_sb[:].opt(), x_d[c].opt())
                if c == 1:
                    # consts: needed from the transpose/matmul stage onwards
                    id_sb = pc.tile([128, 128], BF16)
                    nc.sync.dma_start(id_sb[:], id_d[:])
                    w_sb = pc.tile([128, NDC, 2 * H], BF16)
                    nc.sync.dma_start(w_sb[:].opt(), w_d[:].opt())
                    if not fused:
                        bias_sb = pc.tile([128, 12], F32)
                        nc.sync.dma_start(bias_sb[:], b_d[:])
                mv = pstat.tile([128, 2, NSUB], F32, tag="mv")
                for i in range(NSUB):
                    bn6 = ptmp.tile([128, 6], F32, tag="bn6")
                    nc.vector.bn_stats(bn6[:], x_sb[:, i, :])
                    nc.vector.bn_aggr(mv[:, :, i], bn6[:])
                # rstd = 1/sqrt(var+eps), one Newton step to wash out the
                # ACT sqrt table's loose tolerance
                ve = ptmp.tile([128, NSUB], F32, tag="ve")
                nc.vector.tensor_scalar(ve[:], mv[:, 1, :], LN_EPS, None, OP.add)
                sq = ptmp.tile([128, NSUB], F32, tag="sq")
                nc.scalar.activation(sq[:], ve[:], AF.Sqrt)
                r0 = ptmp.tile([128, NSUB], F32, tag="r0")
                nc.vector.reciprocal(r0[:], sq[:])
                rr = ptmp.tile([128, NSUB], F32, tag="rr")
                nc.vector.tensor_tensor(rr[:], r0[:], r0[:], OP.mult)
                vr = ptmp.tile([128, NSUB], F32, tag="vr")
                nc.vector.tensor_tensor(vr[:], rr[:], ve[:], OP.mult)
                t3 = ptmp.tile([128, NSUB], F32, tag="t3")
                nc.vector.tensor_scalar(t3[:], vr[:], -0.5, 1.5, OP.mult, OP.add)
                r1 = pstat.tile([128, NSUB], F32, tag="r1")
                nc.vector.tensor_tensor(r1[:], t3[:], r0[:], OP.mult)
                xs.append(x_sb)
                stats.append((mv, r1))

            # ---- Phase B: per-chunk pipeline (out-stage deferred one
            # chunk so PE prioritizes the next chunk's matmuls over
            # back-transposes) ----
            hTs = []

            def emit_out_stage(c):
                x_sb = xs[c]
                hT = hTs[c]
                outt = pout.tile([128, NSUB, TC], F32, tag="out")
                for grp in range(2):
                    pb = pbk.tile([128, 2, D], BF16, tag="bk")
                    for u in range(2):
                        i = 2 * grp + u
                        for j in range(4):
                            nc.tensor.transpose(
                                pb[:, u, j * 128:(j + 1) * 128],
                                hT[:, j, i * 128:(i + 1) * 128],
                                id_sb[:],
                            )
                    sl = slice(2 * grp, 2 * grp + 2)
                    nc.vector.tensor_tensor(outt[:, sl, :], pb[:], x_sb[:, sl, :],
                                            OP.add)
                    if c == NCHUNK - 1 and grp == 1:
                        hid_sb = pc.tile([32, H], F32)
                        nc.scalar.copy(hid_sb[:], pb[96:128, 1, :])
                        nc.sync.dma_start(hid_d[:], hid_sb[31:32, :])
                    if c == NCHUNK - 1:
                        nc.sync.dma_start(out_d[c][:, sl, :].opt(),
                                          outt[:, sl, :].opt())
                if c < NCHUNK - 1:
                    nc.sync.dma_start(out_d[c].opt(), outt[:].opt())

            for c in range(NCHUNK):
                x_sb = xs[c]
                mv, r1 = stats[c]

                xn = pxn.tile([128, NSUB, TC], BF16, tag="xn")
                for i in range(NSUB):
                    nc.gpsimd.tensor_scalar(
                        xn[:, i, :], x_sb[:, i, :],
                        mv[:, 0, i:i + 1], r1[:, i:i + 1],
                        OP.subtract, OP.mult,
                    )

                xnT = pxnT.tile([128, NDC, TC], BF16, tag="xnT")
                for dc in range(NDC):
                    pt = ptr.tile([128, TC], BF16, tag="ptr")
                    for i in range(NSUB):
                        nc.tensor.transpose(
                            pt[:, i * 128:(i + 1) * 128],
                            xn[:, i, dc * 128:(dc + 1) * 128],
                            id_sb[:],
                        )
                    nc.scalar.copy(xnT[:, dc, :], pt[:])

                cb = pcv.tile([128, 4, TC], BF16, tag="c")
                sb_ = pcv.tile([128, 4, TC], BF16, tag="s")
                gb = pcv.tile([128, 4, TC], BF16, tag="g")
                vp = pcv.tile([128, 4, TC], BF16, tag="vp")

                if fused:
                    # beta == 0: biases are constants, so post-matmul ops fuse
                    # across hc pairs ([128, 1024] spanning 2 PSUM banks)
                    for half in range(2):
                        km = pmm.tile([128, 2, TC], F32, tag="mm")
                        for u in range(2):
                            hc = 2 * half + u
                            for dc in range(NDC):
                                nc.tensor.matmul(
                                    km[:, u, :],
                                    w_sb[:, dc, hc * 128:(hc + 1) * 128],
                                    xnT[:, dc, :],
                                    start=(dc == 0), stop=(dc == NDC - 1),
                                )
                        sl = slice(2 * half, 2 * half + 2)
                        nc.scalar.activation(cb[:, sl, :], km[:], AF.Sigmoid,
                                             scale=-1.0)
                    for half in range(2):
                        hm = pmm.tile([128, 2, TC], F32, tag="mm")
                        for u in range(2):
                            hc = 4 + 2 * half + u
                            for dc in range(NDC):
                                nc.tensor.matmul(
                                    hm[:, u, :],
                                    w_sb[:, dc, hc * 128:(hc + 1) * 128],
                                    xnT[:, dc, :],
                                    start=(dc == 0), stop=(dc == NDC - 1),
                                )
                        sl = slice(2 * half, 2 * half + 2)
                        nc.scalar.activation(sb_[:, sl, :], hm[:], AF.Sigmoid)
                        nc.vector.scalar_tensor_tensor(
                            gb[:, sl, :], hm[:], 0.5, sb_[:, sl, :],
                            OP.add, OP.max,
                        )
                        cm1 = pcv.tile([128, 2, TC], BF16, tag="cm1")
                        nc.vector.tensor_scalar(
                            cm1[:], cb[:, sl, :], 1.0, None, OP.subtract)
                        nc.vector.tensor_tensor(
                            vp[:, sl, :], cm1[:], gb[:, sl, :], OP.mult)
                else:
                    for hc in range(NHC):
                        ps = pmm.tile([128, 2, TC], F32, tag="mm")
                        for dc in range(NDC):
                            nc.tensor.matmul(
                                ps[:, 0, :],
                                w_sb[:, dc, hc * 128:(hc + 1) * 128],
                                xnT[:, dc, :],
                                start=(dc == 0), stop=(dc == NDC - 1),
                            )
                        j = hc % 4
                        if hc < 4:
                            nc.scalar.activation(
                                cb[:, j, :], ps[:, 0, :], AF.Sigmoid,
                                bias=bias_sb[:, j:j + 1], scale=-1.0,
                            )
                        else:
                            nc.scalar.activation(
                                sb_[:, j, :], ps[:, 0, :], AF.Sigmoid,
                                bias=bias_sb[:, 4 + j:5 + j], scale=1.0,
                            )
                            nc.vector.scalar_tensor_tensor(
                                gb[:, j, :], ps[:, 0, :], bias_sb[:, 8 + j:9 + j],
                                sb_[:, j, :], OP.add, OP.max,
                            )
                            nc.vector.scalar_tensor_tensor(
                                vp[:, j, :], cb[:, j, :], 1.0,
                                gb[:, j, :], OP.subtract, OP.mult,
                            )

                hT = ph.tile([128, 4, TC], BF16, tag="h")
                for j in range(4):
                    init = 0.0 if c == 0 else hTs[c - 1][:, j, TC - 1:TC]
                    nc.vector.tensor_tensor_scan(
                        hT[:, j, :], cb[:, j, :], vp[:, j, :], init,
                        op0=OP.mult, op1=OP.subtract,
                    )
                hTs.append(hT)
                if c >= 1:
                    emit_out_stage(c - 1)
                if c == NCHUNK - 1:
                    emit_out_stage(c)

    nc.compile()
    return nc


_PROGRAMS = {}


def _get_program(fused=True):
    if fused not in _PROGRAMS:
        _PROGRAMS[fused] = _build(fused)
    return _PROGRAMS[fused]


def _permute_x(xc):
    # [4096, 512] -> [NCHUNK, 128, NSUB, TC] with row t = c*512 + i*128 + p
    return np.ascontiguousarray(
        xc.reshape(NCHUNK, NSUB, 128, D).transpose(0, 2, 1, 3)
    )


def _unpermute_out(oc):
    # [NCHUNK, 128, NSUB, TC] -> [4096, 512]
    return oc.transpose(0, 2, 1, 3).reshape(T, D)


def _host_prep(x, ln_gamma, ln_beta, W_gh):
    x = np.ascontiguousarray(np.asarray(x, dtype=np.float32))
    g = np.asarray(ln_gamma, dtype=np.float32)
    be = np.asarray(ln_beta, dtype=np.float32)
    W = np.asarray(W_gh, dtype=np.float32)
    Wp = (g[:, None] * W).astype(ml_dtypes.bfloat16)
    Wp = np.ascontiguousarray(Wp.reshape(NDC, 128, 2 * H).transpose(1, 0, 2))
    s2 = (be @ W).astype(np.float32)  # [2H]
    bias = np.zeros((128, 12), dtype=np.float32)
    for j in range(4):
        bias[:, j] = -s2[j * 128:(j + 1) * 128]
        bias[:, 4 + j] = s2[H + j * 128: H + (j + 1) * 128]
        bias[:, 8 + j] = s2[H + j * 128: H + (j + 1) * 128] + 0.5
    ident = np.eye(128, dtype=ml_dtypes.bfloat16)
    identf = np.eye(128, dtype=np.float32)
    fused = bool(np.all(s2 == 0.0))
    return x, Wp, bias, ident, identf, fused


def kernel(x, ln_gamma, ln_beta, W_gh, _trace=False, _spmd_kwargs=None):
    x, Wp, bias, ident, identf, fused = _host_prep(x, ln_gamma, ln_beta, W_gh)
    nc = _get_program(fused)
    in_maps = [
        {"x": _permute_x(x[c]), "w": Wp, "bias": bias, "ident": ident,
         "identf": identf}
        for c in range(NCORES)
    ]
    kw = dict(_spmd_kwargs or {})
    if _trace:
        kw.setdefault("trace", True)
    res = run_bass_kernel_spmd(nc, in_maps, list(range(NCORES)), **kw)
    out = np.stack([_unpermute_out(res.results[c]["out"]) for c in range(NCORES)])
    hidden = np.stack([res.results[c]["hidden"] for c in range(NCORES)])
    if _trace:
        return (out, hidden), res
    return out, hidden


# revision 26
# speedup vs baseline: 1.0694x; 1.0209x over previous
"""MinGRU block kernel for Trainium2 (8 NeuronCores, batch-parallel).

Reference computation (per batch sample, sharded one sample per core):
  xn = LayerNorm(x) * gamma + beta
  gh = xn @ W_gh ;  k = gh[:, :H], h_pre = gh[:, H:]
  log-space Heinsen scan == direct linear recurrence (exact algebraic identity):
      z = sigmoid(k); g = where(h_pre>=0, h_pre+0.5, sigmoid(h_pre))
        = max(h_pre+0.5, sigmoid(h_pre))          # exact: curves cross only at 0
      h_t = (1-z_t) h_{t-1} + z_t g_t
  out = h + x ; hidden = h[:, -1]

Device mapping per core:
  - LayerNorm stats via bn_stats/bn_aggr (DVE), rsqrt via sqrt+reciprocal+Newton
  - xn cast to bf16 (GPSIMD), transposed 128x128 on PE into [d, t] layout
  - GEMM W'^T @ xn^T in bf16 (W' = gamma-folded W), K=4x128 PSUM accumulation
  - c = sigmoid(-k) (ACT from PSUM), g = max(psum+0.5, s) fused stt (DVE)
  - v' = (c-1)*g (DVE); hardware scan: state = (c * state) - v'  (DVE, f32 out)
  - back-transpose h on PE (f32), residual add from PSUM (DVE), DMA out
gamma/beta are folded host-side: W' = gamma[:,None]*W, s2 = beta @ W rides the
activation biases (zero for the reference inputs, enabling 2-bank fused ops).
Host pre-permutes x/out/W DRAM layouts so DMAs are contiguous per partition.
"""

import numpy as np
import ml_dtypes

import concourse.bass as bass
import concourse.bacc as bacc
import concourse.mybir as mybir
import concourse.tile as tile
from concourse.bass_utils import run_bass_kernel_spmd

B, T, D, H = 8, 4096, 512, 512
NCORES = 8
NCHUNK = 8          # t-chunks per core
TC = T // NCHUNK    # 512 timesteps per chunk
NSUB = TC // 128    # 4 sub-tiles of 128 rows
NDC = D // 128      # 4 contraction chunks
NHC = (2 * H) // 128  # 8 output-channel chunks
F32 = mybir.dt.float32
BF16 = mybir.dt.bfloat16
LN_EPS = 1e-10
AF = mybir.ActivationFunctionType
OP = mybir.AluOpType


def _build(fused: bool):
    nc = bacc.Bacc("TRN2", target_bir_lowering=False, debug=False, num_devices=NCORES)
    # host-permuted layouts: x/out [c, p, i, d] with row t = c*512 + i*128 + p
    x_d = nc.declare_dram_parameter("x", [NCHUNK, 128, NSUB, TC], F32, isOutput=False)
    w_d = nc.declare_dram_parameter("w", [128, NDC, 2 * H], BF16, isOutput=False)
    b_d = nc.declare_dram_parameter("bias", [128, 12], F32, isOutput=False)
    id_d = nc.declare_dram_parameter("ident", [128, 128], BF16, isOutput=False)
    idf_d = nc.declare_dram_parameter("identf", [128, 128], F32, isOutput=False)
    out_d = nc.declare_dram_parameter(
        "out", [NCHUNK, 128, NSUB, TC], F32, isOutput=True
    )
    hid_d = nc.declare_dram_parameter("hidden", [H], F32, isOutput=True)

    with tile.TileContext(nc) as tc:
        with (
            tc.tile_pool(name="const", bufs=1) as pc,
            tc.tile_pool(name="px", bufs=NCHUNK) as px,
            tc.tile_pool(name="pstat", bufs=NCHUNK) as pstat,
            tc.tile_pool(name="ptmp", bufs=3) as ptmp,
            tc.tile_pool(name="pxn", bufs=3) as pxn,
            tc.tile_pool(name="pxnT", bufs=3) as pxnT,
            tc.tile_pool(name="pcv", bufs=3) as pcv,
            tc.tile_pool(name="ph", bufs=3) as ph,
            tc.tile_pool(name="pout", bufs=2) as pout,
            tc.tile_pool(name="ptr", bufs=2, space="PSUM") as ptr,
            tc.tile_pool(name="pmm", bufs=2, space="PSUM") as pmm,
            tc.tile_pool(name="pbk", bufs=2, space="PSUM") as pbk,
        ):
            # ---- Phase A: load all x, LayerNorm stats ----
            # (x chunk 0 is issued first, split per subtile, so DVE work can
            # start ~6us earlier; the 1MB weight load rides behind it)
            xs, stats = [], []
            w_sb = None
            for c in range(NCHUNK):
                x_sb = px.tile([128, NSUB, TC], F32, tag="x")
                if c == 0:
                    for i in range(NSUB):
                        nc.sync.dma_start(x_sb[:, i, :], x_d[c][:, i, :])
                else:
                    nc.sync.dma_start(x_sb[:].opt(), x_d[c].opt())
                if c == 1:
                    # consts: needed from the transpose/matmul stage onwards
                    id_sb = pc.tile([128, 128], BF16)
                    nc.sync.dma_start(id_sb[:], id_d[:])
                    w_sb = pc.tile([128, NDC, 2 * H], BF16)
                    nc.sync.dma_start(w_sb[:].opt(), w_d[:].opt())
                    if not fused:
                        bias_sb = pc.tile([128, 12], F32)
                        nc.sync.dma_start(bias_sb[:], b_d[:])
                mv = pstat.tile([128, 2, NSUB], F32, tag="mv")
                for i in range(NSUB):
                    bn6 = ptmp.tile([128, 6], F32, tag="bn6")
                    nc.vector.bn_stats(bn6[:], x_sb[:, i, :])
                    nc.vector.bn_aggr(mv[:, :, i], bn6[:])
                # rstd = 1/sqrt(var+eps), one Newton step to wash out the
                # ACT sqrt table's loose tolerance
                ve = ptmp.tile([128, NSUB], F32, tag="ve")
                nc.gpsimd.tensor_scalar(ve[:], mv[:, 1, :], LN_EPS, None, OP.add)
                sq = ptmp.tile([128, NSUB], F32, tag="sq")
                nc.scalar.activation(sq[:], ve[:], AF.Sqrt)
                r0 = ptmp.tile([128, NSUB], F32, tag="r0")
                nc.vector.reciprocal(r0[:], sq[:])
                rr = ptmp.tile([128, NSUB], F32, tag="rr")
                nc.gpsimd.tensor_tensor(rr[:], r0[:], r0[:], OP.mult)
                vr = ptmp.tile([128, NSUB], F32, tag="vr")
                nc.gpsimd.tensor_tensor(vr[:], rr[:], ve[:], OP.mult)
                t3 = ptmp.tile([128, NSUB], F32, tag="t3")
                nc.gpsimd.tensor_scalar(t3[:], vr[:], -0.5, 1.5, OP.mult, OP.add)
                r1 = pstat.tile([128, NSUB], F32, tag="r1")
                nc.gpsimd.tensor_tensor(r1[:], t3[:], r0[:], OP.mult)
                xs.append(x_sb)
                stats.append((mv, r1))

            # ---- Phase B: per-chunk pipeline (out-stage deferred one
            # chunk so PE prioritizes the next chunk's matmuls over
            # back-transposes) ----
            hTs = []

            def emit_out_stage(c):
                x_sb = xs[c]
                hT = hTs[c]
                outt = pout.tile([128, NSUB, TC], F32, tag="out")
                for grp in range(2):
                    pb = pbk.tile([128, 2, D], BF16, tag="bk")
                    for u in range(2):
                        i = 2 * grp + u
                        for j in range(4):
                            nc.tensor.transpose(
                                pb[:, u, j * 128:(j + 1) * 128],
                                hT[:, j, i * 128:(i + 1) * 128],
                                id_sb[:],
                            )
                    sl = slice(2 * grp, 2 * grp + 2)
                    nc.vector.tensor_tensor(outt[:, sl, :], pb[:], x_sb[:, sl, :],
                                            OP.add)
                    if c == NCHUNK - 1 and grp == 1:
                        hid_sb = pc.tile([32, H], F32)
                        nc.scalar.copy(hid_sb[:], pb[96:128, 1, :])
                        nc.sync.dma_start(hid_d[:], hid_sb[31:32, :])
                    if c == NCHUNK - 1:
                        nc.sync.dma_start(out_d[c][:, sl, :].opt(),
                                          outt[:, sl, :].opt())
                if c < NCHUNK - 1:
                    nc.sync.dma_start(out_d[c].opt(), outt[:].opt())

            for c in range(NCHUNK):
                x_sb = xs[c]
                mv, r1 = stats[c]

                xn = pxn.tile([128, NSUB, TC], BF16, tag="xn")
                for i in range(NSUB):
                    nc.gpsimd.tensor_scalar(
                        xn[:, i, :], x_sb[:, i, :],
                        mv[:, 0, i:i + 1], r1[:, i:i + 1],
                        OP.subtract, OP.mult,
                    )

                xnT = pxnT.tile([128, NDC, TC], BF16, tag="xnT")
                for dc in range(NDC):
                    pt = ptr.tile([128, TC], BF16, tag="ptr")
                    for i in range(NSUB):
                        nc.tensor.transpose(
                            pt[:, i * 128:(i + 1) * 128],
                            xn[:, i, dc * 128:(dc + 1) * 128],
                            id_sb[:],
                        )
                    nc.scalar.copy(xnT[:, dc, :], pt[:])

                cb = pcv.tile([128, 4, TC], BF16, tag="c")
                sb_ = pcv.tile([128, 4, TC], BF16, tag="s")
                gb = pcv.tile([128, 4, TC], BF16, tag="g")
                vp = pcv.tile([128, 4, TC], BF16, tag="vp")

                if fused:
                    # beta == 0: biases are constants, so post-matmul ops fuse
                    # across hc pairs ([128, 1024] spanning 2 PSUM banks)
                    for half in range(2):
                        km = pmm.tile([128, 2, TC], F32, tag="mm")
                        for u in range(2):
                            hc = 2 * half + u
                            for dc in range(NDC):
                                nc.tensor.matmul(
                                    km[:, u, :],
                                    w_sb[:, dc, hc * 128:(hc + 1) * 128],
                                    xnT[:, dc, :],
                                    start=(dc == 0), stop=(dc == NDC - 1),
                                )
                        sl = slice(2 * half, 2 * half + 2)
                        nc.scalar.activation(cb[:, sl, :], km[:], AF.Sigmoid,
                                             scale=-1.0)
                    for half in range(2):
                        hm = pmm.tile([128, 2, TC], F32, tag="mm")
                        for u in range(2):
                            hc = 4 + 2 * half + u
                            for dc in range(NDC):
                                nc.tensor.matmul(
                                    hm[:, u, :],
                                    w_sb[:, dc, hc * 128:(hc + 1) * 128],
                                    xnT[:, dc, :],
                                    start=(dc == 0), stop=(dc == NDC - 1),
                                )
                        sl = slice(2 * half, 2 * half + 2)
                        nc.scalar.activation(sb_[:, sl, :], hm[:], AF.Sigmoid)
                        nc.vector.scalar_tensor_tensor(
                            gb[:, sl, :], hm[:], 0.5, sb_[:, sl, :],
                            OP.add, OP.max,
                        )
                        cm1 = pcv.tile([128, 2, TC], BF16, tag="cm1")
                        nc.vector.tensor_scalar(
                            cm1[:], cb[:, sl, :], 1.0, None, OP.subtract)
                        nc.vector.tensor_tensor(
                            vp[:, sl, :], cm1[:], gb[:, sl, :], OP.mult)
                else:
                    for hc in range(NHC):
                        ps = pmm.tile([128, 2, TC], F32, tag="mm")
                        for dc in range(NDC):
                            nc.tensor.matmul(
                                ps[:, 0, :],
                                w_sb[:, dc, hc * 128:(hc + 1) * 128],
                                xnT[:, dc, :],
                                start=(dc == 0), stop=(dc == NDC - 1),
                            )
                        j = hc % 4
                        if hc < 4:
                            nc.scalar.activation(
                                cb[:, j, :], ps[:, 0, :], AF.Sigmoid,
                                bias=bias_sb[:, j:j + 1], scale=-1.0,
                            )
                        else:
                            nc.scalar.activation(
                                sb_[:, j, :], ps[:, 0, :], AF.Sigmoid,
                                bias=bias_sb[:, 4 + j:5 + j], scale=1.0,
                            )
                            nc.vector.scalar_tensor_tensor(
                                gb[:, j, :], ps[:, 0, :], bias_sb[:, 8 + j:9 + j],
                                sb_[:, j, :], OP.add, OP.max,
                            )
                            nc.vector.scalar_tensor_tensor(
                                vp[:, j, :], cb[:, j, :], 1.0,
                                gb[:, j, :], OP.subtract, OP.mult,
                            )

                hT = ph.tile([128, 4, TC], BF16, tag="h")
                for j in range(4):
                    init = 0.0 if c == 0 else hTs[c - 1][:, j, TC - 1:TC]
                    nc.vector.tensor_tensor_scan(
                        hT[:, j, :], cb[:, j, :], vp[:, j, :], init,
                        op0=OP.mult, op1=OP.subtract,
                    )
                hTs.append(hT)
                if c >= 1:
                    emit_out_stage(c - 1)
                if c == NCHUNK - 1:
                    emit_out_stage(c)

    nc.compile()
    return nc


_PROGRAMS = {}


def _get_program(fused=True):
    if fused not in _PROGRAMS:
        _PROGRAMS[fused] = _build(fused)
    return _PROGRAMS[fused]


def _permute_x(xc):
    # [4096, 512] -> [NCHUNK, 128, NSUB, TC] with row t = c*512 + i*128 + p
    return np.ascontiguousarray(
        xc.reshape(NCHUNK, NSUB, 128, D).transpose(0, 2, 1, 3)
    )


def _unpermute_out(oc):
    # [NCHUNK, 128, NSUB, TC] -> [4096, 512]
    return oc.transpose(0, 2, 1, 3).reshape(T, D)


def _host_prep(x, ln_gamma, ln_beta, W_gh):
    x = np.ascontiguousarray(np.asarray(x, dtype=np.float32))
    g = np.asarray(ln_gamma, dtype=np.float32)
    be = np.asarray(ln_beta, dtype=np.float32)
    W = np.asarray(W_gh, dtype=np.float32)
    Wp = (g[:, None] * W).astype(ml_dtypes.bfloat16)
    Wp = np.ascontiguousarray(Wp.reshape(NDC, 128, 2 * H).transpose(1, 0, 2))
    s2 = (be @ W).astype(np.float32)  # [2H]
    bias = np.zeros((128, 12), dtype=np.float32)
    for j in range(4):
        bias[:, j] = -s2[j * 128:(j + 1) * 128]
        bias[:, 4 + j] = s2[H + j * 128: H + (j + 1) * 128]
        bias[:, 8 + j] = s2[H + j * 128: H + (j + 1) * 128] + 0.5
    ident = np.eye(128, dtype=ml_dtypes.bfloat16)
    identf = np.eye(128, dtype=np.float32)
    fused = bool(np.all(s2 == 0.0))
    return x, Wp, bias, ident, identf, fused


def kernel(x, ln_gamma, ln_beta, W_gh, _trace=False, _spmd_kwargs=None):
    x, Wp, bias, ident, identf, fused = _host_prep(x, ln_gamma, ln_beta, W_gh)
    nc = _get_program(fused)
    in_maps = [
        {"x": _permute_x(x[c]), "w": Wp, "bias": bias, "ident": ident,
         "identf": identf}
        for c in range(NCORES)
    ]
    kw = dict(_spmd_kwargs or {})
    if _trace:
        kw.setdefault("trace", True)
    res = run_bass_kernel_spmd(nc, in_maps, list(range(NCORES)), **kw)
    out = np.stack([_unpermute_out(res.results[c]["out"]) for c in range(NCORES)])
    hidden = np.stack([res.results[c]["hidden"] for c in range(NCORES)])
    if _trace:
        return (out, hidden), res
    return out, hidden
